# revision 12
# baseline (speedup 1.0000x reference)
"""v14: fast head/tail (v13) + compact producer groups (v12 lesson).

Changes vs v12 (365.7us):
- Warmup matmuls on a memset scratch tile start right after the
  framework preamble (~6.3us) and ramp the PE clock while the first
  DMAs land; x0 pieces ride at the FRONT of all three DMA queues.
- DMA issue order is strict need-time order; the long tail of
  late-need weights rides on sync (idle until out-DMAs begin).
- Producer qk groups are emitted as 2 compact units of 6 matmuls
  (rope with the 2nd): fine enough to absorb the ACT exp latency,
  compact enough that the 2-buffer psum-acc rotation never waits on
  late rope reads (v13's 2-matmul spreading caused multi-us convoy
  stalls once per chunk).
- o-projection emitted as per-lt closures (9 matmuls), spread across
  the carrier chunk.
- Tail: last chunk (2,3) split 384+128 cols so the 384-col norm and
  its o units overlap the 128-col attention; keep-warm matmuls bridge
  the final norm so the last o units run at full clock; f16 output
  DMA (partials summed on host in f32) halves the output drain.
"""

import os
import sys

for _p in ("/opt/trn_rl_repo", "/root/.axon_site/_ro/trn_rl_repo"):
    if os.path.isdir(_p) and _p not in sys.path:
        sys.path.insert(0, _p)

import contextlib

import numpy as np

import concourse.bass as bass
import concourse.tile as tile
from concourse import bacc, mybir
from concourse.bass_utils import run_bass_kernel_spmd

P = 128
L = 2048
D = 1536
HL = 6
HD = 64
EQ = 384
NQK = 768
DC = D // P      # 12
LT = L // P      # 16
ACH = 512
NCQ = L // ACH   # 4
F32 = mybir.dt.float32
F16 = mybir.dt.float16
AF = mybir.ActivationFunctionType

N_WARM = 66        # prologue warmup matmuls (N=512); ends ~x0 arrival (~23us)
N_WARM_TAIL = 22   # keep-warm matmuls bridging the final norm


def build_bass(repeat=1):
    nc = bacc.Bacc("TRN2", target_bir_lowering=False, debug=False, num_devices=8)
    xT = nc.dram_tensor("xT", [D, L], F16, kind="ExternalInput")
    wqkT = nc.dram_tensor("wqkT", [D, NQK], F16, kind="ExternalInput")
    wvT = nc.dram_tensor("wvT", [D, EQ], F16, kind="ExternalInput")
    woT = nc.dram_tensor("woT", [EQ, D], F16, kind="ExternalInput")
    cos2 = nc.dram_tensor("cos2", [P, L], F16, kind="ExternalInput")
    ss2 = nc.dram_tensor("ss2", [P, L], F16, kind="ExternalInput")
    out = nc.dram_tensor("out", [L, D], F16, kind="ExternalOutput")

    xT_r = xT.rearrange("(dc p) l -> p dc l", p=P)
    wqkT_r = wqkT.rearrange("(dc p) e -> p dc e", p=P)
    wvT_r = wvT.rearrange("(dc p) e -> p dc e", p=P)
    woT_r = woT.rearrange("(ec p) d -> p ec d", p=P)

    with tile.TileContext(nc) as tc:
        rep_cm = tc.For_i(0, repeat, 1) if repeat > 1 else contextlib.nullcontext()
        with rep_cm, tc.tile_pool(name="persist", bufs=1) as persist:
            xsb = persist.tile([P, DC, L], F16)
            qT = persist.tile([P, 3, L], F16)
            kT = persist.tile([P, 3, L], F16)
            v1 = persist.tile([P, LT, HL, HD + 1], F16)
            cos_sb = persist.tile([P, L], F16)
            ss_sb = persist.tile([P, L], F16)
            outT = persist.tile([P, 3, L], F16)
            wqks_all = persist.tile([P, DC, 3, 2, P], F16)  # [dc, etp, q/k, 128]
            wv_sb = persist.tile([P, DC, EQ], F16)
            wo_sb = persist.tile([P, 3, D], F16)
            wup = persist.tile([P, ACH], F16)

            def wqk_dma(eng, etp, half):
                base = EQ if half else 0
                eng.dma_start(
                    wqks_all[:, :, etp, half, :],
                    wqkT_r[:, :, base + etp * P : base + (etp + 1) * P],
                )

            def wv_dma(eng, d0):
                eng.dma_start(wv_sb[:, d0 : d0 + 3, :], wvT_r[:, d0 : d0 + 3, :])

            def cs_dma(eng, c):
                sl = slice(c * ACH, (c + 1) * ACH)
                eng.dma_start(cos_sb[:, sl], cos2[:, sl])
                eng.dma_start(ss_sb[:, sl], ss2[:, sl])

            def x_piece(eng, c, pc):
                sl = slice(c * ACH, (c + 1) * ACH)
                eng.dma_start(xsb[:, 3 * pc : 3 * pc + 3, sl], xT_r[:, 3 * pc : 3 * pc + 3, sl])

            # scratch for warmup: no DMA dependency, PE can start ~6.3us in
            nc.gpsimd.memset(wup[:], 0.125)

            # --- DMA issues in strict need-time order. Only sync, scalar
            # and gpsimd can issue DMAs; scalar must be free by the first
            # exp (~17us) and gpsimd by the first rope add (~17us), so the
            # long tail of late-need transfers rides on sync (idle until
            # the out-DMAs begin ~230us in).
            x_piece(nc.sync, 0, 0)
            x_piece(nc.scalar, 0, 1)
            x_piece(nc.gpsimd, 0, 2)
            x_piece(nc.gpsimd, 0, 3)
            wqk_dma(nc.sync, 0, 1)
            wqk_dma(nc.scalar, 0, 0)
            wv_dma(nc.sync, 0)
            wv_dma(nc.scalar, 3)
            wv_dma(nc.gpsimd, 6)
            wv_dma(nc.gpsimd, 9)
            cs_dma(nc.scalar, 0)
            x_piece(nc.sync, 1, 0)
            x_piece(nc.gpsimd, 1, 1)
            x_piece(nc.gpsimd, 1, 2)
            x_piece(nc.scalar, 1, 3)
            cs_dma(nc.scalar, 1)
            x_piece(nc.sync, 2, 0)
            x_piece(nc.gpsimd, 2, 1)
            x_piece(nc.gpsimd, 2, 2)
            x_piece(nc.sync, 2, 3)
            x_piece(nc.sync, 3, 0)
            x_piece(nc.gpsimd, 3, 1)
            x_piece(nc.gpsimd, 3, 2)
            x_piece(nc.sync, 3, 3)
            wqk_dma(nc.sync, 1, 1)
            wqk_dma(nc.sync, 1, 0)
            cs_dma(nc.sync, 2)
            cs_dma(nc.sync, 3)
            wqk_dma(nc.sync, 2, 1)
            wqk_dma(nc.sync, 2, 0)
            nc.sync.dma_start(wo_sb[:], woT_r[:])

            ones_c = nc.const_aps.tensor(1.0, (P, 1), F32)
            nc.vector.tensor_copy(
                v1[:, :, :, HD : HD + 1], ones_c.to_broadcast([P, LT, HL, 1])
            )

            with (
                tc.tile_pool(name="s2t", bufs=2) as s2t,
                tc.tile_pool(name="s2att", bufs=2) as s2att,
                tc.tile_pool(name="s2o", bufs=3) as s2o,
                tc.tile_pool(name="s2nrm", bufs=3) as s2nrm,
                tc.tile_pool(name="ps_acc", bufs=2, space=bass.MemorySpace.PSUM) as ps_acc,
                tc.tile_pool(name="ps_s", bufs=2, space=bass.MemorySpace.PSUM) as ps_s,
                tc.tile_pool(name="ps_av", bufs=2, space=bass.MemorySpace.PSUM) as ps_av,
            ):

                def rope_store(ps, etp, c, half):
                    sl = slice(c * ACH, (c + 1) * ACH)
                    dst = (qT if half == 0 else kT)[:, etp, sl]
                    tcos = s2t.tile([P, ACH], F32, tag="tcos")
                    trot = s2t.tile([P, ACH], F32, tag="trot")
                    nc.vector.tensor_mul(tcos[:], ps[:], cos_sb[:, sl])
                    for q_ in range(4):
                        s = (q_ ^ 1) * 32
                        d_ = q_ * 32
                        nc.vector.tensor_mul(
                            trot[d_ : d_ + 32, :],
                            ps[s : s + 32, :],
                            ss_sb[d_ : d_ + 32, sl],
                        )
                    nc.vector.tensor_add(dst, tcos[:], trot[:])

                def qk_units(etp, c, half):
                    """2 units of 6 accumulating matmuls; 2nd unit ropes.
                    Short psum lifetime (~2 slots) keeps the 2-buffer acc
                    rotation clear of the rope's vector reads."""
                    sl = slice(c * ACH, (c + 1) * ACH)
                    box = {}

                    def mk(i):
                        def f():
                            if i == 0:
                                box["ps"] = ps_acc.tile([P, ACH], F32, tag="acc", name="qkacc")
                            ps = box["ps"]
                            for dc in range(6 * i, 6 * i + 6):
                                nc.tensor.matmul(
                                    ps[:],
                                    wqks_all[:, dc, etp, half, :],
                                    xsb[:, dc, sl],
                                    start=(dc == 0),
                                    stop=(dc == DC - 1),
                                )
                            if i == 1:
                                rope_store(ps, etp, c, half)

                        return f

                    return [mk(i) for i in range(2)]

                def qk_group(etp, c, half):
                    for u in qk_units(etp, c, half):
                        u()

                def v_group(lk):
                    pv = ps_acc.tile([P, ACH], F32, tag="acc")
                    for dc in range(DC):
                        nc.tensor.matmul(
                            pv[:, 0:EQ],
                            xsb[:, dc, lk * P : (lk + 1) * P],
                            wv_sb[:, dc, :],
                            start=(dc == 0),
                            stop=(dc == DC - 1),
                        )
                    nc.scalar.copy(
                        v1[:, lk, :, 0:HD],
                        pv[:, 0:EQ].rearrange("p (h d) -> p h d", h=HL),
                    )

                def o_lt(lt_abs):
                    l0 = lt_abs * P
                    for dn in range(3):
                        pso = ps_acc.tile([P, ACH], F32, tag="acc")
                        for ec in range(3):
                            nc.tensor.matmul(
                                pso[:],
                                outT[:, ec, l0 : l0 + P],
                                wo_sb[:, ec, dn * ACH : (dn + 1) * ACH],
                                start=(ec == 0),
                                stop=(ec == 2),
                            )
                        ot = s2o.tile([P, ACH], F16)
                        nc.vector.tensor_copy(ot[:], pso[:])
                        oeng = nc.sync if (lt_abs + dn) % 2 == 0 else nc.gpsimd
                        oeng.dma_start(
                            out[l0 : l0 + P, dn * ACH : (dn + 1) * ACH], ot[:]
                        )

                def o_units(cq, lts=(0, 1, 2, 3)):
                    return [(lambda lt=lt: o_lt(cq * 4 + lt)) for lt in lts]

                def attention_cq(etp, q0, qw, fillers):
                    """One attention chunk over q columns [q0, q0+qw).
                    fillers: list of LT lists of closures, emitted between
                    the exp and the AV pair of each lk (PE chews them while
                    ACT computes the exp)."""
                    cqs = slice(q0, q0 + qw)
                    pav0 = ps_av.tile([HD + 1, ACH], F32, tag="av")
                    pav1 = ps_av.tile([HD + 1, ACH], F32, tag="av")
                    for lk in range(LT):
                        # head hh lives at column offset hh*ACH: a matmul
                        # output must not cross a 2KB psum bank boundary.
                        pscore = ps_s.tile([P, 2 * ACH], F32, tag="s")
                        att = s2att.tile([P, 2 * ACH], F16)
                        for hh in range(2):  # row-tiled pair, concurrent
                            po = hh * HD
                            nc.tensor.matmul(
                                pscore[:, hh * ACH : hh * ACH + qw],
                                kT[po : po + HD, etp, lk * P : (lk + 1) * P],
                                qT[po : po + HD, etp, cqs],
                                start=True,
                                stop=True,
                            )
                        if qw == ACH:
                            nc.scalar.activation(
                                att[:], pscore[:], AF.Exp, scale=0.125
                            )
                        else:
                            for hh in range(2):
                                nc.scalar.activation(
                                    att[:, hh * ACH : hh * ACH + qw],
                                    pscore[:, hh * ACH : hh * ACH + qw],
                                    AF.Exp,
                                    scale=0.125,
                                )
                        for f in fillers[lk]:
                            f()
                        for hh, pav in ((0, pav0), (1, pav1)):
                            nc.tensor.matmul(
                                pav[:, 0:qw],
                                v1[:, lk, 2 * etp + hh, :],
                                att[:, hh * ACH : hh * ACH + qw],
                                start=(lk == 0),
                                stop=(lk == LT - 1),
                            )
                    # normalization: stage to SBUF, reciprocal, broadcast, mul
                    souts, rcps, rbs = [], [], []
                    for hh, pav in ((0, pav0), (1, pav1)):
                        sout = s2nrm.tile([HD + 1, ACH], F32, tag="sout")
                        nc.vector.tensor_copy(sout[:, 0:qw], pav[:, 0:qw])
                        dcp = s2nrm.tile([1, ACH], F32, tag="dcp")
                        nc.vector.tensor_copy(dcp[:, 0:qw], sout[HD : HD + 1, 0:qw])
                        rcp = s2nrm.tile([1, ACH], F32, tag="rcp")
                        nc.vector.reciprocal_approx_fast(out=rcp[:, 0:qw], in_=dcp[:, 0:qw])
                        souts.append(sout)
                        rcps.append(rcp)
                    for hh in range(2):
                        rb = s2nrm.tile([HD, ACH], F32, tag="rb")
                        nc.gpsimd.partition_broadcast(rb[:, 0:qw], rcps[hh][:, 0:qw], channels=HD)
                        rbs.append(rb)
                    for hh in range(2):
                        po = hh * HD
                        nc.vector.tensor_mul(
                            outT[po : po + HD, etp, cqs],
                            souts[hh][0:HD, 0:qw],
                            rbs[hh][:, 0:qw],
                        )

                def vg(lk):
                    return lambda lk=lk: v_group(lk)

                def spread(units, lo=0, hi=LT):
                    """Distribute units evenly over lk slots [lo, hi)."""
                    slots = [[] for _ in range(LT)]
                    n = len(units)
                    w = hi - lo
                    for i, u in enumerate(units):
                        slots[lo + (i * w) // n].append(u)
                    return slots

                # --- PE warmup on the memset tile: starts right after the
                # preamble (no DMA dependency), ramps the clock while the
                # first weight/x transfers land.
                wps = [ps_s.tile([P, ACH], F32, tag="s", name=f"warm{i}") for i in range(2)]
                for i in range(N_WARM):
                    nc.tensor.matmul(
                        wps[i % 2][:], wup[:, 0:P], wup[:], start=True, stop=True
                    )

                # --- minimal prologue: k chunk 0, q chunk 0 (ropes need ~3us
                # on DVE+Pool, so both groups go before the first v tiles to
                # hide that latency), then v tiles 0-1.
                qk_group(0, 0, 1)
                qk_group(0, 0, 0)
                v_group(0)
                v_group(1)

                # --- chunk (0,0): JIT weave. vg(k) must land before AV lk=k,
                # k-chunk c before scores lk=4c, all x-DMA-paced. qk groups
                # stay WHOLE here (interleaving another group's psum-acc
                # allocation mid-vg would corrupt the rotation).
                f00 = [[] for _ in range(LT)]
                f00[0] = [vg(2), vg(3)]
                for k in range(4, LT):
                    f00[k - 2].append(vg(k))
                f00[1].insert(0, lambda: qk_group(0, 1, 1))   # k chunk1 by lk4
                f00[5].insert(0, lambda: qk_group(0, 2, 1))   # k chunk2 by lk8
                f00[9].insert(0, lambda: qk_group(0, 3, 1))   # k chunk3 by lk12
                f00[13].append(lambda: qk_group(0, 1, 0))     # q chunk1 by (0,1)

                # --- remaining producer groups, deadline-ordered, spread
                # evenly inside their carrier chunk. (e,c,h): h=1 keys are
                # needed at lk=4c of every chunk of pair e; h=0 queries at
                # lk0 of chunk (e,c).
                def qg(etp, c, half):
                    return qk_units(etp, c, half)

                plan = {
                    (0, 1): qg(0, 2, 0) + qg(1, 0, 1) + qg(0, 3, 0),
                    (0, 2): qg(1, 1, 1) + qg(1, 0, 0) + qg(1, 2, 1),
                    (0, 3): qg(1, 3, 1) + qg(1, 1, 0),
                    (1, 0): qg(1, 2, 0) + qg(2, 0, 1),
                    (1, 1): qg(1, 3, 0) + qg(2, 1, 1),
                    (1, 2): qg(2, 2, 1) + qg(2, 0, 0),
                    (1, 3): qg(2, 3, 1) + qg(2, 1, 0),
                    (2, 0): qg(2, 2, 0) + qg(2, 3, 0),
                }

                attention_cq(0, 0, ACH, f00)
                for cq in range(1, NCQ):
                    attention_cq(0, cq * ACH, ACH, spread(plan[(0, cq)]))
                for cq in range(NCQ):
                    attention_cq(1, cq * ACH, ACH, spread(plan[(1, cq)]))
                attention_cq(2, 0, ACH, spread(plan[(2, 0)]))
                attention_cq(2, ACH, ACH, spread(o_units(0), lo=2))
                attention_cq(2, 2 * ACH, ACH, spread(o_units(1), lo=2))
                # last chunk split 384+128: the 384-col norm + its o units
                # overlap the 128-col attention; only a 128-col norm and 3 o
                # units remain after the last AV.
                attention_cq(2, 3 * ACH, 384, spread(o_units(2), lo=2))
                f23b = spread(o_units(3, lts=(0, 1, 2)), lo=5, hi=14)
                attention_cq(2, 3 * ACH + 384, 128, f23b)

                # keep the PE clock up while the final 128-col norm runs
                for i in range(N_WARM_TAIL):
                    nc.tensor.matmul(
                        wps[i % 2][:], wup[:, 0:P], wup[:], start=True, stop=True
                    )
                for u in o_units(3, lts=(3,)):
                    u()

    nc.compile()
    return nc


_NC_CACHE = None


def _get_nc():
    global _NC_CACHE
    if _NC_CACHE is None:
        _NC_CACHE = build_bass()
    return _NC_CACHE


def make_in_maps(x, w_qkv, w_o, cos, sin):
    x = np.asarray(x, dtype=np.float32)
    w_qkv = np.asarray(w_qkv, dtype=np.float32)
    w_o = np.asarray(w_o, dtype=np.float32)
    cos = np.asarray(cos, dtype=np.float32)
    sin = np.asarray(sin, dtype=np.float32)

    cosT = np.ascontiguousarray(cos.T)
    sinT = sin.T
    ss = np.concatenate([-sinT[0:32], sinT[32:64]], axis=0)
    cos2 = np.ascontiguousarray(np.tile(cosT, (2, 1))).astype(np.float16)
    ss2 = np.ascontiguousarray(np.tile(ss, (2, 1))).astype(np.float16)

    in_maps = []
    for c in range(8):
        b, g = c // 4, c % 4
        xTc = np.ascontiguousarray(x[b].T).astype(np.float16)
        wq = w_qkv[g * EQ : (g + 1) * EQ]
        wk = w_qkv[D + g * EQ : D + (g + 1) * EQ]
        wv = w_qkv[2 * D + g * EQ : 2 * D + (g + 1) * EQ]
        wqkTc = np.ascontiguousarray(np.concatenate([wq, wk], 0).T).astype(np.float16)
        wvTc = np.ascontiguousarray(wv.T).astype(np.float16)
        woTc = np.ascontiguousarray(w_o[:, g * EQ : (g + 1) * EQ].T).astype(np.float16)
        in_maps.append(
            {
                "xT": xTc,
                "wqkT": wqkTc,
                "wvT": wvTc,
                "woT": woTc,
                "cos2": cos2,
                "ss2": ss2,
            }
        )
    return in_maps


def kernel(x, w_qkv, w_o, cos, sin):
    nc = _get_nc()
    in_maps = make_in_maps(x, w_qkv, w_o, cos, sin)
    res = run_bass_kernel_spmd(nc, in_maps, core_ids=list(range(8)))
    outs = [res.results[c]["out"].astype(np.float32) for c in range(8)]
    full = np.stack(
        [
            outs[0] + outs[1] + outs[2] + outs[3],
            outs[4] + outs[5] + outs[6] + outs[7],
        ]
    )
    return full


# revision 14
# speedup vs baseline: 1.0012x; 1.0012x over previous
"""v14: fast head/tail (v13) + compact producer groups (v12 lesson).

Changes vs v12 (365.7us):
- Warmup matmuls on a memset scratch tile start right after the
  framework preamble (~6.3us) and ramp the PE clock while the first
  DMAs land; x0 pieces ride at the FRONT of all three DMA queues.
- DMA issue order is strict need-time order; the long tail of
  late-need weights rides on sync (idle until out-DMAs begin).
- Producer qk groups are emitted as 2 compact units of 6 matmuls
  (rope with the 2nd): fine enough to absorb the ACT exp latency,
  compact enough that the 2-buffer psum-acc rotation never waits on
  late rope reads (v13's 2-matmul spreading caused multi-us convoy
  stalls once per chunk).
- o-projection emitted as per-lt closures (9 matmuls), spread across
  the carrier chunk.
- Tail: last chunk (2,3) split 384+128 cols so the 384-col norm and
  its o units overlap the 128-col attention; keep-warm matmuls bridge
  the final norm so the last o units run at full clock; f16 output
  DMA (partials summed on host in f32) halves the output drain.
"""

import os
import sys

for _p in ("/opt/trn_rl_repo", "/root/.axon_site/_ro/trn_rl_repo"):
    if os.path.isdir(_p) and _p not in sys.path:
        sys.path.insert(0, _p)

import contextlib

import numpy as np

import concourse.bass as bass
import concourse.tile as tile
from concourse import bacc, mybir
from concourse.bass_utils import run_bass_kernel_spmd

P = 128
L = 2048
D = 1536
HL = 6
HD = 64
EQ = 384
NQK = 768
DC = D // P      # 12
LT = L // P      # 16
ACH = 512
NCQ = L // ACH   # 4
F32 = mybir.dt.float32
F16 = mybir.dt.float16
AF = mybir.ActivationFunctionType

N_WARM = 22        # prologue warmup matmuls; ~863ns apart they bridge to x0 (~26us)
N_WARM_TAIL = 4    # keep-warm matmuls bridging the final norm


def build_bass(repeat=1):
    nc = bacc.Bacc("TRN2", target_bir_lowering=False, debug=False, num_devices=8)
    xT = nc.dram_tensor("xT", [D, L], F16, kind="ExternalInput")
    wqkT = nc.dram_tensor("wqkT", [D, NQK], F16, kind="ExternalInput")
    wvT = nc.dram_tensor("wvT", [D, EQ], F16, kind="ExternalInput")
    woT = nc.dram_tensor("woT", [EQ, D], F16, kind="ExternalInput")
    cos2 = nc.dram_tensor("cos2", [P, L], F16, kind="ExternalInput")
    ss2 = nc.dram_tensor("ss2", [P, L], F16, kind="ExternalInput")
    out = nc.dram_tensor("out", [L, D], F16, kind="ExternalOutput")

    xT_r = xT.rearrange("(dc p) l -> p dc l", p=P)
    wqkT_r = wqkT.rearrange("(dc p) e -> p dc e", p=P)
    wvT_r = wvT.rearrange("(dc p) e -> p dc e", p=P)
    woT_r = woT.rearrange("(ec p) d -> p ec d", p=P)

    with tile.TileContext(nc) as tc:
        rep_cm = tc.For_i(0, repeat, 1) if repeat > 1 else contextlib.nullcontext()
        with rep_cm, tc.tile_pool(name="persist", bufs=1) as persist:
            xsb = persist.tile([P, DC, L], F16)
            qT = persist.tile([P, 3, L], F16)
            kT = persist.tile([P, 3, L], F16)
            v1 = persist.tile([P, LT, HL, HD + 1], F16)
            cos_sb = persist.tile([P, L], F16)
            ss_sb = persist.tile([P, L], F16)
            outT = persist.tile([P, 3, L], F16)
            wqks_all = persist.tile([P, DC, 3, 2, P], F16)  # [dc, etp, q/k, 128]
            wv_sb = persist.tile([P, DC, EQ], F16)
            wo_sb = persist.tile([P, 3, D], F16)
            wup = persist.tile([P, ACH], F16)

            def wqk_dma(eng, etp, half):
                base = EQ if half else 0
                eng.dma_start(
                    wqks_all[:, :, etp, half, :],
                    wqkT_r[:, :, base + etp * P : base + (etp + 1) * P],
                )

            def wv_dma(eng, d0):
                eng.dma_start(wv_sb[:, d0 : d0 + 3, :], wvT_r[:, d0 : d0 + 3, :])

            def cs_dma(eng, c):
                sl = slice(c * ACH, (c + 1) * ACH)
                eng.dma_start(cos_sb[:, sl], cos2[:, sl])
                eng.dma_start(ss_sb[:, sl], ss2[:, sl])

            def x_piece(eng, c, pc):
                sl = slice(c * ACH, (c + 1) * ACH)
                eng.dma_start(xsb[:, 3 * pc : 3 * pc + 3, sl], xT_r[:, 3 * pc : 3 * pc + 3, sl])

            # scratch for warmup: no DMA dependency, PE can start ~6.3us in
            nc.gpsimd.memset(wup[:], 0.125)

            # --- DMA issues in strict need-time order. Only sync, scalar
            # and gpsimd can issue DMAs; scalar must be free by the first
            # exp (~17us) and gpsimd by the first rope add (~17us), so the
            # long tail of late-need transfers rides on sync (idle until
            # the out-DMAs begin ~230us in).
            x_piece(nc.sync, 0, 0)
            x_piece(nc.scalar, 0, 1)
            x_piece(nc.gpsimd, 0, 2)
            x_piece(nc.gpsimd, 0, 3)
            wqk_dma(nc.sync, 0, 1)
            wqk_dma(nc.scalar, 0, 0)
            wv_dma(nc.sync, 0)
            wv_dma(nc.scalar, 3)
            wv_dma(nc.gpsimd, 6)
            wv_dma(nc.gpsimd, 9)
            cs_dma(nc.scalar, 0)
            x_piece(nc.sync, 1, 0)
            x_piece(nc.gpsimd, 1, 1)
            x_piece(nc.gpsimd, 1, 2)
            x_piece(nc.scalar, 1, 3)
            cs_dma(nc.scalar, 1)
            x_piece(nc.sync, 2, 0)
            x_piece(nc.gpsimd, 2, 1)
            x_piece(nc.gpsimd, 2, 2)
            x_piece(nc.sync, 2, 3)
            x_piece(nc.sync, 3, 0)
            x_piece(nc.gpsimd, 3, 1)
            x_piece(nc.gpsimd, 3, 2)
            x_piece(nc.sync, 3, 3)
            wqk_dma(nc.sync, 1, 1)
            wqk_dma(nc.sync, 1, 0)
            cs_dma(nc.sync, 2)
            cs_dma(nc.sync, 3)
            wqk_dma(nc.sync, 2, 1)
            wqk_dma(nc.sync, 2, 0)
            nc.sync.dma_start(wo_sb[:], woT_r[:])

            ones_c = nc.const_aps.tensor(1.0, (P, 1), F32)
            nc.vector.tensor_copy(
                v1[:, :, :, HD : HD + 1], ones_c.to_broadcast([P, LT, HL, 1])
            )

            with (
                tc.tile_pool(name="s2t", bufs=2) as s2t,
                tc.tile_pool(name="s2att", bufs=2) as s2att,
                tc.tile_pool(name="s2o", bufs=3) as s2o,
                tc.tile_pool(name="s2nrm", bufs=3) as s2nrm,
                tc.tile_pool(name="ps_acc", bufs=2, space=bass.MemorySpace.PSUM) as ps_acc,
                tc.tile_pool(name="ps_s", bufs=2, space=bass.MemorySpace.PSUM) as ps_s,
                tc.tile_pool(name="ps_av", bufs=2, space=bass.MemorySpace.PSUM) as ps_av,
            ):

                def rope_store(ps, etp, c, half):
                    sl = slice(c * ACH, (c + 1) * ACH)
                    dst = (qT if half == 0 else kT)[:, etp, sl]
                    tcos = s2t.tile([P, ACH], F32, tag="tcos")
                    trot = s2t.tile([P, ACH], F32, tag="trot")
                    nc.vector.tensor_mul(tcos[:], ps[:], cos_sb[:, sl])
                    for q_ in range(4):
                        s = (q_ ^ 1) * 32
                        d_ = q_ * 32
                        nc.vector.tensor_mul(
                            trot[d_ : d_ + 32, :],
                            ps[s : s + 32, :],
                            ss_sb[d_ : d_ + 32, sl],
                        )
                    nc.vector.tensor_add(dst, tcos[:], trot[:])

                def qk_units(etp, c, half):
                    """2 units of 6 accumulating matmuls; 2nd unit ropes.
                    Short psum lifetime (~2 slots) keeps the 2-buffer acc
                    rotation clear of the rope's vector reads."""
                    sl = slice(c * ACH, (c + 1) * ACH)
                    box = {}

                    def mk(i):
                        def f():
                            if i == 0:
                                box["ps"] = ps_acc.tile([P, ACH], F32, tag="acc", name="qkacc")
                            ps = box["ps"]
                            for dc in range(6 * i, 6 * i + 6):
                                nc.tensor.matmul(
                                    ps[:],
                                    wqks_all[:, dc, etp, half, :],
                                    xsb[:, dc, sl],
                                    start=(dc == 0),
                                    stop=(dc == DC - 1),
                                )
                            if i == 1:
                                rope_store(ps, etp, c, half)

                        return f

                    return [mk(i) for i in range(2)]

                def qk_group(etp, c, half):
                    for u in qk_units(etp, c, half):
                        u()

                def v_group(lk):
                    pv = ps_acc.tile([P, ACH], F32, tag="acc")
                    for dc in range(DC):
                        nc.tensor.matmul(
                            pv[:, 0:EQ],
                            xsb[:, dc, lk * P : (lk + 1) * P],
                            wv_sb[:, dc, :],
                            start=(dc == 0),
                            stop=(dc == DC - 1),
                        )
                    nc.scalar.copy(
                        v1[:, lk, :, 0:HD],
                        pv[:, 0:EQ].rearrange("p (h d) -> p h d", h=HL),
                    )

                def o_lt(lt_abs):
                    l0 = lt_abs * P
                    for dn in range(3):
                        pso = ps_acc.tile([P, ACH], F32, tag="acc")
                        for ec in range(3):
                            nc.tensor.matmul(
                                pso[:],
                                outT[:, ec, l0 : l0 + P],
                                wo_sb[:, ec, dn * ACH : (dn + 1) * ACH],
                                start=(ec == 0),
                                stop=(ec == 2),
                            )
                        ot = s2o.tile([P, ACH], F16)
                        nc.vector.tensor_copy(ot[:], pso[:])
                        oeng = nc.sync if (lt_abs + dn) % 2 == 0 else nc.gpsimd
                        oeng.dma_start(
                            out[l0 : l0 + P, dn * ACH : (dn + 1) * ACH], ot[:]
                        )

                def o_units(cq, lts=(0, 1, 2, 3)):
                    return [(lambda lt=lt: o_lt(cq * 4 + lt)) for lt in lts]

                def attention_cq(etp, q0, qw, fillers, att_sink=None):
                    """One attention chunk over q columns [q0, q0+qw).
                    fillers: list of LT lists of closures, emitted between
                    the exp and the AV pair of each lk (PE chews them while
                    ACT computes the exp)."""
                    cqs = slice(q0, q0 + qw)
                    pav0 = ps_av.tile([HD + 1, ACH], F32, tag="av")
                    pav1 = ps_av.tile([HD + 1, ACH], F32, tag="av")
                    for lk in range(LT):
                        # head hh lives at column offset hh*ACH: a matmul
                        # output must not cross a 2KB psum bank boundary.
                        pscore = ps_s.tile([P, 2 * ACH], F32, tag="s")
                        att = s2att.tile([P, 2 * ACH], F16)
                        for hh in range(2):  # row-tiled pair, concurrent
                            po = hh * HD
                            nc.tensor.matmul(
                                pscore[:, hh * ACH : hh * ACH + qw],
                                kT[po : po + HD, etp, lk * P : (lk + 1) * P],
                                qT[po : po + HD, etp, cqs],
                                start=True,
                                stop=True,
                            )
                        if qw == ACH:
                            nc.scalar.activation(
                                att[:], pscore[:], AF.Exp, scale=0.125
                            )
                        else:
                            for hh in range(2):
                                nc.scalar.activation(
                                    att[:, hh * ACH : hh * ACH + qw],
                                    pscore[:, hh * ACH : hh * ACH + qw],
                                    AF.Exp,
                                    scale=0.125,
                                )
                        for f in fillers[lk]:
                            f()
                        for hh, pav in ((0, pav0), (1, pav1)):
                            nc.tensor.matmul(
                                pav[:, 0:qw],
                                v1[:, lk, 2 * etp + hh, :],
                                att[:, hh * ACH : hh * ACH + qw],
                                start=(lk == 0),
                                stop=(lk == LT - 1),
                            )
                        if att_sink is not None and lk == LT - 1:
                            att_sink["att"] = att
                    # normalization: stage to SBUF, reciprocal, broadcast, mul
                    souts, rcps, rbs = [], [], []
                    for hh, pav in ((0, pav0), (1, pav1)):
                        sout = s2nrm.tile([HD + 1, ACH], F32, tag="sout")
                        nc.vector.tensor_copy(sout[:, 0:qw], pav[:, 0:qw])
                        dcp = s2nrm.tile([1, ACH], F32, tag="dcp")
                        nc.vector.tensor_copy(dcp[:, 0:qw], sout[HD : HD + 1, 0:qw])
                        rcp = s2nrm.tile([1, ACH], F32, tag="rcp")
                        nc.vector.reciprocal_approx_fast(out=rcp[:, 0:qw], in_=dcp[:, 0:qw])
                        souts.append(sout)
                        rcps.append(rcp)
                    for hh in range(2):
                        rb = s2nrm.tile([HD, ACH], F32, tag="rb")
                        nc.gpsimd.partition_broadcast(rb[:, 0:qw], rcps[hh][:, 0:qw], channels=HD)
                        rbs.append(rb)
                    for hh in range(2):
                        po = hh * HD
                        nc.vector.tensor_mul(
                            outT[po : po + HD, etp, cqs],
                            souts[hh][0:HD, 0:qw],
                            rbs[hh][:, 0:qw],
                        )

                def vg(lk):
                    return lambda lk=lk: v_group(lk)

                def spread(units, lo=0, hi=LT):
                    """Distribute units evenly over lk slots [lo, hi)."""
                    slots = [[] for _ in range(LT)]
                    n = len(units)
                    w = hi - lo
                    for i, u in enumerate(units):
                        slots[lo + (i * w) // n].append(u)
                    return slots

                # --- PE warmup on the memset tile: starts right after the
                # preamble (no DMA dependency), ramps the clock while the
                # first weight/x transfers land.
                wps = [ps_s.tile([P, ACH], F32, tag="s", name=f"warm{i}") for i in range(2)]
                for i in range(N_WARM):
                    nc.tensor.matmul(
                        wps[i % 2][:], wup[:, 0:P], wup[:], start=True, stop=True
                    )

                # --- minimal prologue: k chunk 0, q chunk 0 (ropes need ~3us
                # on DVE+Pool, so both groups go before the first v tiles to
                # hide that latency), then v tiles 0-1.
                qk_group(0, 0, 1)
                qk_group(0, 0, 0)
                v_group(0)
                v_group(1)

                # --- chunk (0,0): JIT weave. vg(k) must land before AV lk=k,
                # k-chunk c before scores lk=4c, all x-DMA-paced. qk groups
                # stay WHOLE here (interleaving another group's psum-acc
                # allocation mid-vg would corrupt the rotation).
                f00 = [[] for _ in range(LT)]
                f00[0] = [vg(2), vg(3)]
                for k in range(4, LT):
                    f00[k - 2].append(vg(k))
                f00[1].insert(0, lambda: qk_group(0, 1, 1))   # k chunk1 by lk4
                f00[5].insert(0, lambda: qk_group(0, 2, 1))   # k chunk2 by lk8
                f00[9].insert(0, lambda: qk_group(0, 3, 1))   # k chunk3 by lk12
                f00[13].append(lambda: qk_group(0, 1, 0))     # q chunk1 by (0,1)

                # --- remaining producer groups, deadline-ordered, spread
                # evenly inside their carrier chunk. (e,c,h): h=1 keys are
                # needed at lk=4c of every chunk of pair e; h=0 queries at
                # lk0 of chunk (e,c).
                def qg(etp, c, half):
                    return qk_units(etp, c, half)

                plan = {
                    (0, 1): qg(0, 2, 0) + qg(1, 0, 1) + qg(0, 3, 0),
                    (0, 2): qg(1, 1, 1) + qg(1, 0, 0) + qg(1, 2, 1),
                    (0, 3): qg(1, 3, 1) + qg(1, 1, 0),
                    (1, 0): qg(1, 2, 0) + qg(2, 0, 1),
                    (1, 1): qg(1, 3, 0) + qg(2, 1, 1),
                    (1, 2): qg(2, 2, 1) + qg(2, 0, 0),
                    (1, 3): qg(2, 3, 1) + qg(2, 1, 0),
                    (2, 0): qg(2, 2, 0) + qg(2, 3, 0),
                }

                attention_cq(0, 0, ACH, f00)
                for cq in range(1, NCQ):
                    attention_cq(0, cq * ACH, ACH, spread(plan[(0, cq)]))
                for cq in range(NCQ):
                    attention_cq(1, cq * ACH, ACH, spread(plan[(1, cq)]))
                attention_cq(2, 0, ACH, spread(plan[(2, 0)]))
                attention_cq(2, ACH, ACH, spread(o_units(0), lo=4))
                attention_cq(2, 2 * ACH, ACH, spread(o_units(1), lo=4))
                # last chunk split 384+128: the 384-col norm + its o units
                # overlap the 128-col attention; only a 128-col norm and 3 o
                # units remain after the last AV.
                attention_cq(2, 3 * ACH, 384, spread(o_units(2), lo=4))
                f23b = spread(o_units(3, lts=(0, 1, 2)), lo=5, hi=14)
                last_att = {}
                attention_cq(2, 3 * ACH + 384, 128, f23b, att_sink=last_att)

                # keep the PE clock up while the final 128-col norm runs; the
                # rhs is the last chunk's att tile so the scheduler cannot
                # hoist these earlier.
                for i in range(N_WARM_TAIL):
                    wtail = ps_s.tile([P, 256], F32, tag="s", name=f"wtail{i}")
                    nc.tensor.matmul(
                        wtail[:],
                        wup[:, 0:P],
                        last_att["att"][:, 0:256],
                        start=True,
                        stop=True,
                    )
                for u in o_units(3, lts=(3,)):
                    u()

    nc.compile()
    return nc


_NC_CACHE = None


def _get_nc():
    global _NC_CACHE
    if _NC_CACHE is None:
        _NC_CACHE = build_bass()
    return _NC_CACHE


def make_in_maps(x, w_qkv, w_o, cos, sin):
    x = np.asarray(x, dtype=np.float32)
    w_qkv = np.asarray(w_qkv, dtype=np.float32)
    w_o = np.asarray(w_o, dtype=np.float32)
    cos = np.asarray(cos, dtype=np.float32)
    sin = np.asarray(sin, dtype=np.float32)

    cosT = np.ascontiguousarray(cos.T)
    sinT = sin.T
    ss = np.concatenate([-sinT[0:32], sinT[32:64]], axis=0)
    cos2 = np.ascontiguousarray(np.tile(cosT, (2, 1))).astype(np.float16)
    ss2 = np.ascontiguousarray(np.tile(ss, (2, 1))).astype(np.float16)

    in_maps = []
    for c in range(8):
        b, g = c // 4, c % 4
        xTc = np.ascontiguousarray(x[b].T).astype(np.float16)
        wq = w_qkv[g * EQ : (g + 1) * EQ]
        wk = w_qkv[D + g * EQ : D + (g + 1) * EQ]
        wv = w_qkv[2 * D + g * EQ : 2 * D + (g + 1) * EQ]
        wqkTc = np.ascontiguousarray(np.concatenate([wq, wk], 0).T).astype(np.float16)
        wvTc = np.ascontiguousarray(wv.T).astype(np.float16)
        woTc = np.ascontiguousarray(w_o[:, g * EQ : (g + 1) * EQ].T).astype(np.float16)
        in_maps.append(
            {
                "xT": xTc,
                "wqkT": wqkTc,
                "wvT": wvTc,
                "woT": woTc,
                "cos2": cos2,
                "ss2": ss2,
            }
        )
    return in_maps


def kernel(x, w_qkv, w_o, cos, sin):
    nc = _get_nc()
    in_maps = make_in_maps(x, w_qkv, w_o, cos, sin)
    res = run_bass_kernel_spmd(nc, in_maps, core_ids=list(range(8)))
    outs = [res.results[c]["out"].astype(np.float32) for c in range(8)]
    full = np.stack(
        [
            outs[0] + outs[1] + outs[2] + outs[3],
            outs[4] + outs[5] + outs[6] + outs[7],
        ]
    )
    return full
